# revision 20
# baseline (speedup 1.0000x reference)
"""Trainium2 Bass kernel for nn_DRNLayer (8-core n_upper-sharded).

out[i,j,l] = softmax_l( sum_k log( sum_m exp(w[j,k]*logD[m,l]) * P[i,k,m] ) + B[j,l] )

Sharding: n_upper (j) split 8 ways, 16 j per core; softmax axis (q_up) is local,
so no collectives; host concatenates per-core outputs.

Per-core dataflow (v2 — host-precomputed T, no on-chip exp):
  - T[j,k,l,m] = exp(w[j,k]*logD[l,m]) is batch-independent; host computes it
    once in fp32 and ships bf16 tiles [128=(khalf,m), (jh,j8,l)] per k-pair,
    streamed by DMA during the loop (alternating sync/gpsimd queues).
  - PE computes Pw for two k's at once with a block-diagonal bf16 lhsT
    [[P_k^T, 0], [0, P_k'^T]] (K=128, N=512 per jh half) into PSUM.
  - Even k-pairs: ACT takes Ln(pw) directly (PSUM->SBUF, fp32r) and PE
    accumulates the logs into a persistent PSUM bank via identity matmuls.
  - Odd k-pairs: DVE runs a product chain over 4 groups of 8, ACT takes one
    Ln per group, Pool (gpsimd) accumulates the group logs + bias.
  - Tail: DVE merges PSUM log-accumulator + SBUF logsum, folds the two
    k-halves across partitions, then a local softmax over l and DMA out.
"""

import sys

sys.path.insert(0, "/opt/trn_rl_repo")

from contextlib import ExitStack

import ml_dtypes
import numpy as np

import concourse.bacc as bacc
import concourse.bass as bass
import concourse.mybir as mybir
from concourse.bass_utils import run_bass_kernel_spmd
from concourse.tile import TileContext

F32 = mybir.dt.float32
F32R = mybir.dt.float32r
BF16 = mybir.dt.bfloat16
FP16 = mybir.dt.float16
AF = mybir.ActivationFunctionType
ALU = mybir.AluOpType
BF16_NP = ml_dtypes.bfloat16

N_CORES = 8
BATCH = 64  # i
NJ = 16  # j per core
NK = 128  # n_lower (k)
Q = 64  # q_upper == q_lower (l, m)
NKP = 64  # k-pairs: partition half 0 handles k=kp, half 1 handles k=kp+64
KCH = 4  # k-pairs per T-chunk DMA
ACT_TABLE_LN_EXP = 6  # act_info.json index of natural_log_exp_and_others

_NC = None
LAST_RESULTS = None


def _build():
    nc = bacc.Bacc()
    TT_d = nc.declare_dram_parameter("TT", [128, NKP, 1024], FP16, isOutput=False)
    PT_d = nc.declare_dram_parameter("PTD", [128, NKP, 128], FP16, isOutput=False)
    I_d = nc.declare_dram_parameter("I128", [128, 128], FP16, isOutput=False)
    c_d = nc.declare_dram_parameter("comb", [128, BATCH], F32, isOutput=False)
    b_d = nc.declare_dram_parameter("Bfull", [128, NJ * Q], F32, isOutput=False)
    o_d = nc.declare_dram_parameter("out", [BATCH, NJ, Q], F32, isOutput=True)

    with TileContext(nc) as tc, ExitStack() as ctx:
        # keep Exp+Ln resident in one ACT table for the whole kernel
        nc.scalar.add_instruction(
            mybir.InstLoadActFuncSet(
                name=nc.get_next_instruction_name(),
                ins=[],
                outs=[],
                act_func_set_id=ACT_TABLE_LN_EXP,
            )
        )

        consts = ctx.enter_context(tc.tile_pool(name="consts", bufs=1))
        ptbp = ctx.enter_context(tc.tile_pool(name="ptb", bufs=1))
        tpool = ctx.enter_context(tc.tile_pool(name="tt", bufs=6))
        lnpool = ctx.enter_context(tc.tile_pool(name="lnp", bufs=3))
        apool = ctx.enter_context(tc.tile_pool(name="acc", bufs=2))
        gpool = ctx.enter_context(tc.tile_pool(name="glog", bufs=2))
        lpool = ctx.enter_context(tc.tile_pool(name="lsum", bufs=1))
        spool = ctx.enter_context(tc.tile_pool(name="smax", bufs=1))

        # ---------------- constants / inputs ----------------
        # block-diagonal P^T shipped pre-zero-padded; gpsimd queue keeps the
        # sync queue free for the T stream
        I128 = consts.tile([128, 128], FP16)
        nc.gpsimd.dma_start(out=I128, in_=I_d[:, :])
        comb = consts.tile([128, BATCH], F32)
        nc.gpsimd.dma_start(out=comb, in_=c_d[:, :])
        Bfull = consts.tile([128, NJ * Q], F32)
        nc.gpsimd.dma_start(out=Bfull, in_=b_d[:, :])
        # PTB head rides the fast HWDGE queue ahead of the first T chunk;
        # the tail streams while the first k-pairs already execute
        PTB = ptbp.tile([128, NKP, 128], FP16)
        nc.sync.dma_start(out=PTB[:, 0:8, :], in_=PT_d[:, 0:8, :])

        # running log-sum for the grouped (odd) k-pairs
        logsum = lpool.tile([128, NJ * Q], F32)

        # ---------------- main loop over k-pairs ----------------
        with tc.tile_pool(name="pwps", bufs=3, space="PSUM") as ps_pw, tc.tile_pool(
            name="lgps", bufs=1, space="PSUM"
        ) as ps_lg:
            lgacc = ps_lg.tile([128, NJ * Q], F32)
            acc = None
            n_direct = 0
            n_grp = 0
            for kp in range(NKP):
                if kp == 1:
                    nc.sync.dma_start(out=PTB[:, 8:NKP, :], in_=PT_d[:, 8:NKP, :])
                if kp % KCH == 0:
                    Tch = tpool.tile([128, KCH, 2, 512], FP16, tag="tch")
                    q = (kp // KCH) % 2
                    eng = nc.sync if q == 0 else nc.gpsimd
                    eng.dma_start(out=Tch, in_=TT_d[:, kp : kp + KCH, :].rearrange(
                        "p c (a b) -> p c a b", a=2
                    ))
                kc = kp % KCH
                pw = ps_pw.tile([128, 2, 512], F32, tag="pw")
                pw_flat = pw.rearrange("p a b -> p (a b)")
                for jh in range(2):
                    nc.tensor.matmul(
                        out=pw[:, jh, :],
                        lhsT=PTB[:, kp, :],
                        rhs=Tch[:, kc, jh, :],
                        start=True,
                        stop=True,
                    )
                if kp % 2 == 0:
                    # direct path: Ln on ACT, accumulate logs in PSUM via PE
                    lnt = lnpool.tile([128, NJ * Q], FP16, tag="lnt")
                    nc.scalar.activation(out=lnt, in_=pw_flat, func=AF.Ln, scale=0.03125)
                    for jh in range(2):
                        nc.tensor.matmul(
                            out=lgacc[:, jh * 512 : (jh + 1) * 512],
                            lhsT=I128,
                            rhs=lnt[:, jh * 512 : (jh + 1) * 512],
                            start=(n_direct == 0),
                            stop=(kp == NKP - 2),
                            skip_group_check=True,
                        )
                    n_direct += 1
                else:
                    # grouped path: product chain on DVE, one Ln per 8 pairs
                    r = n_grp % 8
                    g = n_grp // 8
                    n_grp += 1
                    if r == 0:
                        acc = apool.tile([128, NJ * Q], F32, tag="acc")
                        nc.vector.tensor_copy(out=acc, in_=pw_flat)
                    else:
                        nc.vector.tensor_tensor(
                            out=acc, in0=pw_flat, in1=acc, op=ALU.mult
                        )
                    if r == 7:
                        if g == 0:
                            nc.scalar.activation(out=logsum, in_=acc, func=AF.Ln)
                            # fold bias in while the loop is still running
                            nc.gpsimd.tensor_tensor(
                                out=logsum, in0=Bfull, in1=logsum, op=ALU.add
                            )
                        elif g == 3:
                            gllast = lpool.tile([128, NJ * Q], F32)
                            nc.scalar.activation(out=gllast, in_=acc, func=AF.Ln)
                        else:
                            gl = gpool.tile([128, NJ * Q], F32, tag="gl")
                            nc.scalar.activation(out=gl, in_=acc, func=AF.Ln)
                            nc.gpsimd.tensor_tensor(
                                out=logsum, in0=gl, in1=logsum, op=ALU.add
                            )

            # ---------------- tail: merge + softmax ----------------
            # tot = lgacc + logsum is off the critical path (ready at kp~62);
            # only gllast's Ln + the accumulating comb matmuls trail the loop.
            tot = spool.tile([128, NJ * Q], F32)
            nc.vector.tensor_tensor(
                out=tot, in0=lgacc, in1=logsum, op=ALU.add
            )
            lg_flat = lgacc[0:BATCH, :]
            NJH = NJ // 2
            for jh in range(2):
                sl = slice(jh * 512, (jh + 1) * 512)
                nc.tensor.matmul(
                    out=lg_flat[:, sl],
                    lhsT=comb,
                    rhs=tot[:, sl],
                    start=True,
                    stop=False,
                    skip_group_check=True,
                )
                nc.tensor.matmul(
                    out=lg_flat[:, sl],
                    lhsT=comb,
                    rhs=gllast[:, sl],
                    start=False,
                    stop=True,
                    skip_group_check=True,
                )
                lgv = lg_flat[:, sl].rearrange("p (a b) -> p a b", a=NJH)
                mx = spool.tile([BATCH, NJH], F32, tag=f"mx{jh}")
                nc.vector.tensor_reduce(mx, lgv, axis=mybir.AxisListType.X, op=ALU.max)
                em = spool.tile([BATCH, NJH, Q], F32, tag=f"em{jh}")
                nc.vector.tensor_tensor(
                    out=em,
                    in0=lgv,
                    in1=mx.unsqueeze(2).broadcast_to([BATCH, NJH, Q]),
                    op=ALU.subtract,
                )
                nc.scalar.activation(out=em, in_=em, func=AF.Exp)
                sm = spool.tile([BATCH, NJH], F32, tag=f"sm{jh}")
                nc.vector.tensor_reduce(sm, em, axis=mybir.AxisListType.X, op=ALU.add)
                rec = spool.tile([BATCH, NJH], F32, tag=f"rec{jh}")
                nc.vector.reciprocal(rec, sm)
                oute = spool.tile([BATCH, NJH, Q], F32, tag=f"oute{jh}")
                nc.vector.tensor_tensor(
                    out=oute,
                    in0=em,
                    in1=rec.unsqueeze(2).broadcast_to([BATCH, NJH, Q]),
                    op=ALU.mult,
                )
                nc.sync.dma_start(out=o_d[:, jh * NJH : (jh + 1) * NJH, :], in_=oute)

    nc.compile()
    return nc


def kernel(P, weight, bias_abs, bias_q, lambda_abs, lambda_q):
    global _NC, LAST_RESULTS
    P = np.asarray(P, dtype=np.float32)
    # PTD[(c,m), kp, (c',i)] = P[i, kp + 64*c, m] on the diagonal blocks
    PTH = (
        P.reshape(BATCH, 2, NKP, Q)
        .transpose(1, 3, 2, 0)
        .astype(np.float16)
    )
    PTD = np.zeros((128, NKP, 128), dtype=np.float16)
    PTD[0:64, :, 0:64] = PTH[0]
    PTD[64:128, :, 64:128] = PTH[1]
    weight = np.asarray(weight, dtype=np.float32)
    bias_abs = np.asarray(bias_abs, dtype=np.float32)
    bias_q = np.asarray(bias_q, dtype=np.float32)
    lambda_abs = np.asarray(lambda_abs, dtype=np.float32)
    lambda_q = np.asarray(lambda_q, dtype=np.float32)

    if _NC is None:
        _NC = _build()

    mv = np.arange(Q, dtype=np.float32) / Q
    logD = -((mv[None, :] - mv[:, None]) ** 2)  # [l, m]
    s = mv[None, :]  # [1, 64]
    I128 = np.eye(128, dtype=np.float16)
    combm = np.concatenate([np.eye(BATCH, dtype=np.float32)] * 2, axis=0)

    in_maps = []
    for c in range(N_CORES):
        jsl = slice(c * NJ, (c + 1) * NJ)
        wsl = weight[jsl, :]  # [16, 128]
        # T[j,k,l,m] = exp(w[j,k]*logD[l,m]); bf16 tiles [(c,m), kp, (jh,j8,l)]
        T = np.exp(wsl[:, :, None, None] * logD[None, None, :, :])
        T6 = T.reshape(2, 8, 2, NKP, Q, Q)  # (jh, j8, c, kp, l, m)
        TTd = np.ascontiguousarray(
            T6.transpose(2, 5, 3, 0, 1, 4).reshape(128, NKP, 1024)
        ).astype(np.float16)
        Bm = -bias_q[jsl] * (s - lambda_q[jsl]) ** 2 - bias_abs[jsl] * np.abs(
            s - lambda_abs[jsl]
        )  # [16, 64]
        Bfull = np.zeros((128, NJ * Q), dtype=np.float32)
        Bfull[0:64, :] = Bm.reshape(1, NJ * Q)
        in_maps.append(
            {
                "TT": TTd,
                "PTD": PTD,
                "I128": I128,
                "comb": combm,
                "Bfull": Bfull,
            }
        )

    LAST_RESULTS = run_bass_kernel_spmd(_NC, in_maps, list(range(N_CORES)))
    return np.concatenate(
        [LAST_RESULTS.results[c]["out"] for c in range(N_CORES)], axis=1
    )
